# revision 2
# baseline (speedup 1.0000x reference)
"""Trainium2 Bass kernel v3 for the dense transformer block (B=4, T=2048, D=1024, H=16).

Sharding (as v2): 8 cores = 4 pairs; pair p owns batch p. Within a pair:
head-split attention (8 heads/core), partial w_o + pairwise ReduceScatter,
then token-split LN2+MLP (1024 tokens per core).

v3 changes (trace+microbench driven):
  - Dedicated PSUM pools: scores(3) / yp(2) / mm(2) / transpose(1) banks, so
    the scores ring no longer gates on exp's ~1.1us latency (v2: 571ns/matmul
    cadence in C; solo matmul cadence is 216ns).
  - pt (exp output) in fp8 with exp bias -4ln2; v in fp8 -> pv via DoubleRow
    pairs (halves pv matmul count). Error sim: attention quantization is
    negligible vs the MLP path.
  - qkT in fp8 (halves its SBUF to 2MB -> wproj prefetch during C3).
  - MLP entirely bf16 (same PE cost as v2's hi/lo fp8 DR, error 1.7e-2->3e-3).
  - x loaded as bf16 (LN1/attn path only; residual path keeps f32 x_res).
  - Batched transpose evacuation (4x[128,128] per PSUM tile, 1 copy).
  - Per-chunk batched softmax denominators: one [8,512] reciprocal per chunk
    instead of 8x [1,512] (3.4us each).
  - fc(chunk0) + wproj-half prefetch interleaved into C3; gelu/exp on ACT,
    copies on DVE, tri-mask stays on DVE.
"""

import sys

sys.path.insert(0, "/opt/trn_rl_repo")

import numpy as np
import ml_dtypes

import concourse.bass as bass
import concourse.tile as tile
from concourse import mybir
from concourse.bass_utils import run_bass_kernel_spmd
from concourse.masks import make_identity

BF16 = mybir.dt.bfloat16
F32 = mybir.dt.float32
FP8 = mybir.dt.float8e4
DR = mybir.MatmulPerfMode.DoubleRow
AF = mybir.ActivationFunctionType
ALU = mybir.AluOpType

S_X = 8.0    # LN1-output fp8 scale
S_W = 32.0   # wqk/wv fp8 scale
S_Q = 64.0   # qkT fp8 scale
S_V = 64.0   # v fp8 scale
S_Y = 8.0    # attention-y fp8 scale
S_WO = 16.0  # wo fp8 scale
EXP_SCALE = 0.125 / (S_Q * S_Q)
EXP_BIAS = -2.772588722239781  # -4 ln 2: pt = exp(s)/16, cancels in normalize

T = 2048
D = 1024
H = 16
HD = 64
HL = 8   # heads per core
P = 128
NT = T // P   # 16
ND = D // P   # 8
TL = T // 2   # tokens owned per core
NTL = TL // P  # 8
FC = 4 * D
NFC = FC // P  # 32
NYT = HL * HD // P  # 4


def _patch_tile_drain():
    from concourse.tile import ScopedClock

    def patched(self, tick_clock, wait_clock):
        nc = self.nc
        probe = nc.sync.nop(nofuse=True)
        wait_clock.add_sem_waits(probe.ins, ScopedClock({None: tick_clock.global_clock}))
        si = probe.ins.sync_info
        waits = list(si.on_wait) if si and si.on_wait else []
        if len(waits) > 1:
            probe.ins.sync_info = mybir.SyncInfo(
                on_wait=waits[:1], on_update=list(si.on_update or [])
            )
            for i in range(1, len(waits)):
                nop = nc.sync.nop(nofuse=True)
                nop.ins.sync_info = mybir.SyncInfo(on_wait=waits[i : i + 1], on_update=[])
        nc.all_engine_barrier()
        popped = nc._tile_sem_poison_stack.pop()
        assert popped is self._sem_poison
        nc.clear_and_free_semaphores(list(self.sems.allocated().values()))
        nc.all_engine_barrier()

    tile.TileContext._drain_and_barrier = patched


_SYNC_WAIT_CAP = 1


def _split_sync_waits(nc, cap=_SYNC_WAIT_CAP):
    cnt = 0
    for f in nc.m.functions:
        for bb in f.blocks:
            new_insts = []
            for inst in bb.instructions:
                si = inst.sync_info
                waits = list(si.on_wait) if si and si.on_wait else []
                if len(waits) > cap:
                    for i in range(0, len(waits) - cap, cap):
                        nop = mybir.InstNoOp(name=f"waitsplit_{cnt}", ins=[], outs=[])
                        cnt += 1
                        nop.engine = inst.engine
                        nop.sync_info = mybir.SyncInfo(
                            on_wait=waits[i : i + cap], on_update=[]
                        )
                        new_insts.append(nop)
                    inst.sync_info = mybir.SyncInfo(
                        on_wait=waits[len(waits) - cap :],
                        on_update=list(si.on_update or []),
                    )
                new_insts.append(inst)
            bb.instructions[:] = new_insts


def _bcast_ap(ap_1d, parts):
    return bass.AP(
        tensor=ap_1d.tensor,
        offset=ap_1d.offset,
        ap=[[0, parts], list(ap_1d.ap[0])],
    )


def _bcast_row(ap_row, parts):
    """[1,N] AP -> [parts,N] AP with partition stride 0."""
    return bass.AP(
        tensor=ap_row.tensor,
        offset=ap_row.offset,
        ap=[[0, parts]] + [list(d) for d in ap_row.ap[1:]],
    )


def _layernorm(nc, pool, x_tile, out, s=1.0):
    """out [128,1024] = (x - mean) * rsqrt(var) * s (eps dropped: var ~= 1).
    rsqrt via 2 Newton steps on DVE."""
    stats = pool.tile([P, 2, 6], F32, tag="ln_stats")
    for sh in range(2):
        nc.vector.bn_stats(out=stats[:, sh, :], in_=x_tile[:, sh * 512 : (sh + 1) * 512])
    mv = pool.tile([P, 2], F32, tag="ln_mv")
    nc.vector.bn_aggr(out=mv[:], in_=stats[:])
    v = mv[:, 1:2]
    y = pool.tile([P, 1], F32, tag="ln_y")
    t = pool.tile([P, 1], F32, tag="ln_t")
    nc.vector.tensor_scalar(out=y[:], in0=v, scalar1=-0.5, scalar2=1.5, op0=ALU.mult, op1=ALU.add)
    for _ in range(2):
        nc.vector.tensor_mul(out=t[:], in0=y[:], in1=y[:])
        nc.vector.tensor_mul(out=t[:], in0=t[:], in1=v)
        nc.vector.tensor_scalar(out=t[:], in0=t[:], scalar1=-0.5, scalar2=1.5, op0=ALU.mult, op1=ALU.add)
        nc.vector.tensor_mul(out=y[:], in0=y[:], in1=t[:])
    if s != 1.0:
        nc.vector.tensor_scalar(out=y[:], in0=y[:], scalar1=float(s), scalar2=None, op0=ALU.mult)
    nc.vector.tensor_scalar(
        out=out[:],
        in0=x_tile[:],
        scalar1=mv[:, 0:1],
        scalar2=y[:],
        op0=ALU.subtract,
        op1=ALU.mult,
    )


def build_kernel():
    nc = bass.Bass()

    x_ext = nc.declare_dram_parameter("x", [T, D], BF16, isOutput=False)
    xres_ext = nc.declare_dram_parameter("x_res", [TL, D], F32, isOutput=False)
    wqk_ext = nc.declare_dram_parameter("wqk", [D, D], FP8, isOutput=False)
    wv_ext = nc.declare_dram_parameter("wv", [D, HL * HD], FP8, isOutput=False)
    wo_ext = nc.declare_dram_parameter("wo", [HL * HD, D], FP8, isOutput=False)
    wfc_ext = nc.declare_dram_parameter("wfc", [D, FC], BF16, isOutput=False)
    wproj_ext = nc.declare_dram_parameter("wproj", [FC, D], BF16, isOutput=False)
    bfc_ext = nc.declare_dram_parameter("bfc", [FC], F32, isOutput=False)
    bproj_ext = nc.declare_dram_parameter("bproj", [D], F32, isOutput=False)
    tri_ext = nc.declare_dram_parameter("tri", [P, P], BF16, isOutput=False)
    sel_ext = nc.declare_dram_parameter("sel", [HL, HL * HD], BF16, isOutput=False)
    sel2_ext = nc.declare_dram_parameter("sel2", [HD + 1, HL * HL], BF16, isOutput=False)
    out_ext = nc.declare_dram_parameter("out", [TL, D], F32, isOutput=True)

    cc_ins = [nc.dram_tensor(f"cc_in{c}", [512, D], BF16) for c in range(4)]
    cc_outs = [nc.dram_tensor(f"cc_out{c}", [256, D], BF16) for c in range(4)]

    x_r = x_ext.rearrange("(t p) d -> p t d", p=P)
    xres_r = xres_ext.rearrange("(t p) d -> p t d", p=P)
    wqk_r = wqk_ext.rearrange("(dt p) c -> p dt c", p=P)
    wv_r = wv_ext.rearrange("(dt p) c -> p dt c", p=P)
    wo_r = wo_ext.rearrange("(yt p) c -> p yt c", p=P)
    wfc_r = wfc_ext.rearrange("(dt p) c -> p dt c", p=P)
    wproj_r = wproj_ext.rearrange("(ft p) c -> p ft c", p=P)
    cc_in_rs = [t.rearrange("(t p) d -> p t d", p=P) for t in cc_ins]
    cc_out_rs = [t.rearrange("(t p) d -> p t d", p=P) for t in cc_outs]
    out_r = out_ext.rearrange("(t p) d -> p t d", p=P)

    with tile.TileContext(nc) as tc:
        with (
            tc.tile_pool(name="singles", bufs=1) as singles,
            tc.tile_pool(name="psS", bufs=3, space="PSUM") as psS,
            tc.tile_pool(name="psY", bufs=2, space="PSUM") as psY,
            tc.tile_pool(name="psM", bufs=2, space="PSUM") as psM,
            tc.tile_pool(name="psT", bufs=1, space="PSUM") as psT,
        ):
            # ---- constants ----
            ident = singles.tile([P, P], BF16)
            make_identity(nc, ident)
            bproj_sb = singles.tile([P, D], F32)
            nc.sync.dma_start(out=bproj_sb[:], in_=_bcast_ap(bproj_ext[:], P))
            bfc_sb = singles.tile([P, NFC], F32)
            nc.sync.dma_start(out=bfc_sb[:], in_=bfc_ext.rearrange("(o p) -> p o", p=P))
            tri_sb = singles.tile([P, P], BF16)
            nc.sync.dma_start(out=tri_sb[:], in_=tri_ext[:])
            sel_sb = singles.tile([HL, HL * HD], BF16)
            nc.sync.dma_start(out=sel_sb[:], in_=sel_ext[:])
            sel2_sb = singles.tile([HD + 1, HL * HL], BF16)
            nc.sync.dma_start(out=sel2_sb[:], in_=sel2_ext[:])
            ones1 = singles.tile([1, HD], BF16)
            nc.vector.memset(ones1[:], 1.0)
            expb = singles.tile([P, 1], F32)
            nc.vector.memset(expb[:], EXP_BIAS)

            hTa = singles.tile([P, ND, TL], BF16)   # LN2(resid)^T
            resid_sb = singles.tile([P, NTL, D], BF16)

            # ---------- helper emitters ----------
            def emit_transpose_batch(src_bf16, dst, dt0, col0, sa):
                """4 PE transposes of src[:, (dt0+k)*128:...] into one psT tile,
                one DVE copy out to dst[:, dt0:dt0+4, col0:col0+128]."""
                tp = psT.tile([P, 512], BF16, tag="tp")
                for k in range(4):
                    nc.tensor.transpose(
                        tp[:, k * P : (k + 1) * P],
                        src_bf16[:, (dt0 + k) * P : (dt0 + k + 1) * P],
                        ident[:],
                    )
                nc.vector.tensor_copy(
                    out=dst[:, dt0 : dt0 + 4, col0 : col0 + P],
                    in_=tp.rearrange("p (a b) -> p a b", a=4),
                )

            def emit_A_tile(qc, tti, xlT, sa, part=None):
                """part=None: whole tile; 0: dma+LN; 1: transposes."""
                tt = 4 * qc + tti
                if part in (None, 0):
                    x_tile = sa.tile([P, D], BF16, tag=f"x_tile{tt%2}", bufs=2)
                    nc.sync.dma_start(out=x_tile[:], in_=x_r[:, tt, :])
                    xl = sa.tile([P, D], BF16, tag=f"xl{tt%2}", bufs=2)
                    _layernorm(nc, sa, x_tile, xl, s=S_X)
                    sa._v3_xl = getattr(sa, "_v3_xl", {})
                    sa._v3_xl[tt] = xl
                if part in (None, 1):
                    xl = sa._v3_xl[tt]
                    emit_transpose_batch(xl, xlT, 0, tti * P, sa)
                if part in (None, 2):
                    xl = sa._v3_xl.pop(tt)
                    emit_transpose_batch(xl, xlT, 4, tti * P, sa)

            def emit_B_ct(qc, ct, xlT, qkT, wqk_sb):
                qp = psM.tile([P, 512], F32, tag="mm")
                for dt in range(0, ND, 2):
                    nc.tensor.matmul(
                        qp[:],
                        lhsT=wqk_sb[:, dt : dt + 2, ct * P : (ct + 1) * P],
                        rhs=xlT[:, dt : dt + 2, :],
                        start=(dt == 0),
                        stop=(dt == ND - 2),
                        perf_mode=DR,
                    )
                dst = qkT[:, ct, qc * 512 : (qc + 1) * 512]
                if qc <= 2:
                    nc.scalar.activation(
                        out=dst, in_=qp[:], func=AF.Copy, scale=S_Q / (S_X * S_W)
                    )
                else:
                    nc.vector.tensor_scalar(
                        out=dst, in0=qp[:], scalar1=S_Q / (S_X * S_W),
                        scalar2=None, op0=ALU.mult,
                    )

            def emit_B_v(qc, tti, xlT, v_sb, wv_sb):
                tt = 4 * qc + tti
                vp = psM.tile([P, 512], F32, tag="mm")
                for dt in range(0, ND, 2):
                    nc.tensor.matmul(
                        vp[:],
                        lhsT=xlT[:, dt : dt + 2, tti * P : (tti + 1) * P],
                        rhs=wv_sb[:, dt : dt + 2, :],
                        start=(dt == 0),
                        stop=(dt == ND - 2),
                        perf_mode=DR,
                    )
                if qc <= 2:
                    nc.scalar.activation(
                        out=v_sb[:, tt // 2, tt % 2, :, 0:HD],
                        in_=vp.rearrange("p (h e) -> p h e", h=HL),
                        func=AF.Copy,
                        scale=S_V / (S_X * S_W),
                    )
                else:
                    nc.vector.tensor_scalar(
                        out=v_sb[:, tt // 2, tt % 2, :, 0:HD],
                        in0=vp.rearrange("p (h e) -> p h e", h=HL),
                        scalar1=S_V / (S_X * S_W),
                        scalar2=None,
                        op0=ALU.mult,
                    )

            def emit_C_head(qc, h, yT, qkT, v_sb, ptm_pool, ptd, ysb_pool,
                            dsb_t, tails, filler_pop):
                po = (h % 2) * HD
                qt_ct = h // 2
                kt_ct = 4 + h // 2
                npair = 2 * (qc + 1)
                yp = psY.tile([HD + 1, 512], F32, tag="yp")

                def q0_of(kt):
                    j = kt - 4 * qc
                    return 128 * j if j > 0 else 0

                def emit_scores_pair(pr):
                    if pr == 2 * qc:
                        pt = ptd[0][h % 3]
                    elif pr == 2 * qc + 1:
                        pt = ptd[1][h % 2]
                    else:
                        pt = ptm_pool.tile([P, 2, 512], FP8, tag="ptm", bufs=5)
                    for half in range(2):
                        kt = 2 * pr + half
                        q0 = q0_of(kt)
                        sp = psS.tile([P, 512], F32, tag="s")
                        nc.tensor.matmul(
                            sp[:, q0:512],
                            lhsT=qkT[po : po + HD, kt_ct, kt * P : (kt + 1) * P],
                            rhs=qkT[po : po + HD, qt_ct, qc * 512 + q0 : (qc + 1) * 512],
                            start=True,
                            stop=True,
                        )
                        nc.scalar.activation(
                            out=pt[:, half, q0:512],
                            in_=sp[:, q0:512],
                            func=AF.Exp,
                            scale=EXP_SCALE,
                            bias=expb[:],
                        )
                        j = kt - 4 * qc
                        if j >= 0:
                            nc.vector.tensor_mul(
                                out=pt[:, half, q0 : q0 + P],
                                in0=pt[:, half, q0 : q0 + P],
                                in1=tri_sb[:],
                            )
                    return pt

                pts = {}
                for pr in range(min(2, npair)):
                    pts[pr] = emit_scores_pair(pr)
                if tails:
                    tails.pop(0)()
                for pr in range(npair):
                    if pr + 2 < npair:
                        pts[pr + 2] = emit_scores_pair(pr + 2)
                    filler_pop()
                    pt = pts.pop(pr)
                    q0min = q0_of(2 * pr)
                    nc.tensor.matmul(
                        yp[:, q0min:512],
                        lhsT=v_sb[:, pr, :, h, 0 : HD + 1],
                        rhs=pt[:, :, q0min:512],
                        start=(pr == 0),
                        stop=(pr == npair - 1),
                        perf_mode=DR,
                    )

                def tail():
                    ysb_t = ysb_pool.tile([HD + 1, 512], BF16, tag="ysb", bufs=8)
                    nc.vector.tensor_copy(out=ysb_t[:], in_=yp[:])
                    chunk_heads.append((h, ysb_t))

                tails.append(tail)

            def emit_chunk_norm(qc, yT, dsb_t, recp):
                dps = psM.tile([P, 512], F32, tag="mm")
                for i, (h, ysb_t) in enumerate(chunk_heads):
                    nc.tensor.matmul(
                        dps[0:HL, :],
                        lhsT=sel2_sb[:, h * HL : (h + 1) * HL],
                        rhs=ysb_t[:],
                        start=(i == 0),
                        stop=(i == len(chunk_heads) - 1),
                    )
                rec = recp.tile([HL, 512], F32, tag="rec")
                nc.vector.reciprocal(out=rec[:], in_=dps[0:HL, :])
                recb = recp.tile([HL, 512], BF16, tag="recb")
                nc.vector.tensor_scalar(
                    out=recb[:], in0=rec[:], scalar1=S_Y / S_V, scalar2=None, op0=ALU.mult
                )
                for h, ysb_t in chunk_heads:
                    po = (h % 2) * HD
                    bp = psM.tile([P, 512], F32, tag="mm")
                    nc.tensor.matmul(
                        bp[0:HD, :],
                        lhsT=sel_sb[:, h * HD : (h + 1) * HD],
                        rhs=recb[:],
                        start=True,
                        stop=True,
                    )
                    nc.vector.tensor_mul(
                        out=yT[po : po + HD, h // 2, :],
                        in0=ysb_t[0:HD, :],
                        in1=bp[0:HD, :],
                    )
                chunk_heads.clear()

            def emit_D_group(pqc, g, sc, wo_sb, yT_chunks):
                yT = yT_chunks[pqc]
                t4, half = g // 2, g % 2
                op = psM.tile([P, 512], F32, tag="mm")
                for yt in range(0, NYT, 2):
                    nc.tensor.matmul(
                        op[:],
                        lhsT=yT[:, yt : yt + 2, t4 * P : (t4 + 1) * P],
                        rhs=wo_sb[:, yt : yt + 2, half * 512 : (half + 1) * 512],
                        start=(yt == 0),
                        stop=(yt == NYT - 2),
                        perf_mode=DR,
                    )
                ob = sc.tile([P, 512], BF16, tag="ob")
                nc.vector.tensor_scalar(
                    out=ob[:],
                    in0=op[:],
                    scalar1=1.0 / (S_Y * S_WO),
                    scalar2=None,
                    op0=ALU.mult,
                )
                nc.sync.dma_start(
                    out=cc_in_rs[pqc][:, t4, half * 512 : (half + 1) * 512],
                    in_=ob[:],
                )

            def emit_RS(pqc):
                nc.gpsimd.collective_compute(
                    "ReduceScatter",
                    ALU.add,
                    ins=[cc_ins[pqc][:]],
                    outs=[cc_outs[pqc][:]],
                    replica_groups=[[0, 1], [2, 3], [4, 5], [6, 7]],
                )

            def emit_E_tile(pqc, i2, se):
                tt = 2 * pqc + i2
                rs_bf = se.tile([P, D], BF16, tag="rs_bf")
                nc.sync.dma_start(out=rs_bf[:], in_=cc_out_rs[pqc][:, i2, :])
                xr = se.tile([P, D], F32, tag="xr")
                nc.sync.dma_start(out=xr[:], in_=xres_r[:, tt, :])
                nc.vector.tensor_add(out=xr[:], in0=xr[:], in1=rs_bf[:])
                nc.vector.tensor_copy(out=resid_sb[:, tt, :], in_=xr[:])
                h2 = se.tile([P, D], BF16, tag="h2")
                _layernorm(nc, se, xr, h2)
                for dt0 in (0, 4):
                    emit_transpose_batch(h2, hTa, dt0, tt * P, se)

            def emit_fc_chain(tc2, fct, sf, gT, wf_tiles, defer_gelu=False):
                """One fc output tile: 8 bf16 matmuls + gelu (or deferred:
                h_mid evacuated bf16 by DVE, gelu'd in-place later so the ACT
                engine never swaps exp<->gelu tables mid-C3)."""
                wf = wf_tiles[fct // 4]
                f1 = fct % 4
                fp = psM.tile([P, 512], F32, tag="mm")
                for dt in range(ND):
                    nc.tensor.matmul(
                        fp[:],
                        lhsT=wf[:, dt, f1 * P : (f1 + 1) * P],
                        rhs=hTa[:, dt, tc2 * 512 : (tc2 + 1) * 512],
                        start=(dt == 0),
                        stop=(dt == ND - 1),
                    )
                if defer_gelu:
                    nc.vector.tensor_copy(out=gT[:, fct, :], in_=fp[:])
                else:
                    nc.scalar.activation(
                        out=gT[:, fct, :],
                        in_=fp[:],
                        func=AF.Gelu_apprx_tanh,
                        bias=bfc_sb[:, fct : fct + 1],
                        scale=1.0,
                    )

            def emit_fc_dma(fc4, sf):
                wf = sf.tile([P, ND, 512], BF16, tag="wf", bufs=2)
                nc.sync.dma_start(
                    out=wf[:], in_=wfc_r[:, :, fc4 * 512 : (fc4 + 1) * 512]
                )
                return wf

            def emit_proj_tile(tc2, t4, half, spool, gT, wp_get):
                tt = tc2 * 4 + t4
                pp = psM.tile([P, 512], F32, tag="mm")
                for fct in range(NFC):
                    rhs, fl = wp_get(half, fct)
                    nc.tensor.matmul(
                        pp[:],
                        lhsT=gT[:, fct, t4 * P : (t4 + 1) * P],
                        rhs=rhs[:, fl, :],
                        start=(fct == 0),
                        stop=(fct == NFC - 1),
                    )
                s1 = spool.tile([P, 512], F32, tag="s1")
                nc.vector.scalar_tensor_tensor(
                    out=s1[:],
                    in0=pp[:],
                    scalar=1.0,
                    in1=resid_sb[:, tt, half * 512 : (half + 1) * 512],
                    op0=ALU.mult,
                    op1=ALU.add,
                )
                nc.vector.tensor_add(
                    out=s1[:],
                    in0=s1[:],
                    in1=bproj_sb[:, half * 512 : (half + 1) * 512],
                )
                nc.sync.dma_start(
                    out=out_r[:, tt, half * 512 : (half + 1) * 512],
                    in_=s1[:],
                )

            # ---------- schedule ----------
            import contextlib
            chunk_heads = []
            yT_chunks = []
            with contextlib.ExitStack() as stk_outer:
                se = stk_outer.enter_context(tc.tile_pool(name="se", bufs=1))
                with contextlib.ExitStack() as stk_mid:
                    pC = stk_mid.enter_context(tc.tile_pool(name="pC", bufs=2))
                    sc = stk_mid.enter_context(tc.tile_pool(name="sc", bufs=3))
                    pW2 = stk_mid.enter_context(tc.tile_pool(name="pW2", bufs=1))
                    sf0 = stk_mid.enter_context(tc.tile_pool(name="sf0", bufs=1))
                    pWp0 = stk_mid.enter_context(tc.tile_pool(name="pWp0", bufs=1))
                    wp0a = pWp0.tile([P, NFC // 2, 512], BF16)
                    pF0 = stk_mid.enter_context(tc.tile_pool(name="pF0", bufs=1))
                    gT0 = pF0.tile([P, NFC, 512], BF16)
                    with contextlib.ExitStack() as stk_kv:
                        pKV = stk_kv.enter_context(tc.tile_pool(name="pKV", bufs=1))
                        ptm_pool = stk_kv.enter_context(tc.tile_pool(name="ptm", bufs=1))
                        recp = stk_kv.enter_context(tc.tile_pool(name="recp", bufs=2))
                        qkT = pKV.tile([P, ND, T], FP8)
                        v_sb = pKV.tile([P, NT // 2, 2, HL, HD + 2], FP8)
                        nc.vector.memset(v_sb[:, :, :, :, HD : HD + 1], 1.0)
                        # dedicated diag pt tiles (dead regions stay zero)
                        ptd = [[None] * 3, [None] * 2]
                        for a in range(2):
                            for b in range(3 - a):
                                t_ = ptm_pool.tile([P, 2, 512], FP8, name=f"ptd{a}{b}", tag=f"ptd{a}{b}")
                                nc.vector.memset(t_[:], 0.0)
                                ptd[a][b] = t_

                        tails = []
                        with contextlib.ExitStack() as stk_a:
                            pW1 = stk_a.enter_context(tc.tile_pool(name="pW1", bufs=1))
                            pA = stk_a.enter_context(tc.tile_pool(name="pA", bufs=1))
                            sa = stk_a.enter_context(tc.tile_pool(name="sa", bufs=2))
                            wqk_sb = pW1.tile([P, ND, D], FP8)
                            wv_sb = pW1.tile([P, ND, HL * HD], FP8)
                            wo_sb = pW2.tile([P, NYT, D], FP8)

                            # prologue: A(0) + B(0)
                            xlT0 = pA.tile([P, ND, 512], tag="xlT", dtype=FP8)
                            emit_A_tile(0, 0, xlT0, sa)
                            nc.sync.dma_start(out=wqk_sb[:], in_=wqk_r[:])
                            nc.sync.dma_start(out=wv_sb[:], in_=wv_r[:])
                            for tti in range(1, 4):
                                emit_A_tile(0, tti, xlT0, sa)
                            nc.sync.dma_start(out=wo_sb[:], in_=wo_r[:])
                            for ct in range(ND):
                                emit_B_ct(0, ct, xlT0, qkT, wqk_sb)
                            for tti in range(4):
                                emit_B_v(0, tti, xlT0, v_sb, wv_sb)

                            # chunks 0..2
                            for qc in range(3):
                                yT = pC.tile([P, NYT, 512], tag="yT", dtype=FP8)
                                yT_chunks.append(yT)
                                xlT_next = pA.tile([P, ND, 512], tag="xlT", dtype=FP8)
                                fillers = []
                                for tti in range(4):
                                    for prt in range(3):
                                        fillers.append(
                                            lambda tti=tti, prt=prt: emit_A_tile(
                                                qc + 1, tti, xlT_next, sa, part=prt
                                            )
                                        )
                                if qc >= 1:
                                    for g in range(8):
                                        fillers.append(
                                            lambda g=g: emit_D_group(qc - 1, g, sc, wo_sb, yT_chunks)
                                        )
                                    fillers.append(lambda: emit_RS(qc - 1))
                                for ct in range(ND):
                                    fillers.append(
                                        lambda ct=ct: emit_B_ct(qc + 1, ct, xlT_next, qkT, wqk_sb)
                                    )
                                if qc >= 2:
                                    for i2 in range(2):
                                        fillers.append(
                                            lambda i2=i2: emit_E_tile(qc - 2, i2, se)
                                        )

                                def filler_pop():
                                    if fillers:
                                        fillers.pop(0)()

                                for h in range(HL):
                                    emit_C_head(
                                        qc, h, yT, qkT, v_sb, ptm_pool, ptd,
                                        ptm_pool, None, tails, filler_pop,
                                    )
                                while tails:
                                    tails.pop(0)()
                                while fillers:
                                    fillers.pop(0)()
                                for tti in range(4):
                                    emit_B_v(qc + 1, tti, xlT_next, v_sb, wv_sb)
                                emit_chunk_norm(qc, yT, None, recp)
                        # C3 with D(2), RS(2), E(1), fc(0), wproj prefetch
                        yT3 = pC.tile([P, NYT, 512], tag="yT", dtype=FP8)
                        yT_chunks.append(yT3)
                        wf_tiles = {}
                        fillers = []
                        fillers.append(
                            lambda: nc.sync.dma_start(
                                out=wp0a[:], in_=wproj_r[:, 0 : NFC // 2, 0:512]
                            )
                        )
                        for g in range(8):
                            fillers.append(lambda g=g: emit_D_group(2, g, sc, wo_sb, yT_chunks))
                        fillers.append(lambda: emit_RS(2))
                        for i2 in range(2):
                            fillers.append(lambda i2=i2: emit_E_tile(1, i2, se))

                        def mk_fcdma(fc4):
                            def f():
                                wf_tiles[fc4] = emit_fc_dma(fc4, sf0)
                            return f

                        def mk_fc(fct):
                            def f():
                                emit_fc_chain(0, fct, sf0, gT0, wf_tiles, defer_gelu=True)
                            return f

                        for fc4 in range(2):
                            fillers.append(mk_fcdma(fc4))
                        for fct in range(NFC):
                            if fct % 4 == 0 and fc4_next(fct) < NFC // 4:
                                fillers.append(mk_fcdma(fc4_next(fct)))
                            fillers.append(mk_fc(fct))

                        def filler_pop3():
                            if fillers:
                                fillers.pop(0)()

                        for h in range(HL):
                            emit_C_head(
                                3, h, yT3, qkT, v_sb, ptm_pool, ptd,
                                ptm_pool, None, tails, filler_pop3,
                            )
                        while tails:
                            tails.pop(0)()
                        emit_chunk_norm(3, yT3, None, recp)
                        while fillers:
                            fillers.pop(0)()
                    # pKV/ptm/recp closed: 52KB freed -> wp0b + wp1
                    pWp1 = stk_mid.enter_context(tc.tile_pool(name="pWp1", bufs=1))
                    wp0b = pWp1.tile([P, NFC // 2, 512], BF16, name="wp0b")
                    nc.sync.dma_start(
                        out=wp0b[:], in_=wproj_r[:, NFC // 2 : NFC, 0:512]
                    )
                    wp1 = pWp1.tile([P, NFC, 512], BF16, name="wp1")
                    nc.sync.dma_start(out=wp1[:], in_=wproj_r[:, :, 512:1024])

                    def wp_get(half, fct):
                        if half == 1:
                            return wp1, fct
                        if fct < NFC // 2:
                            return wp0a, fct
                        return wp0b, fct - NFC // 2

                    # D(3) + RS(3); contiguous in-place gelu pass for fc(0)
                    for g in range(8):
                        emit_D_group(3, g, sc, wo_sb, yT_chunks)
                        if g % 2 == 0:
                            for fct in range(g * 4, g * 4 + 8):
                                nc.scalar.activation(
                                    out=gT0[:, fct, :],
                                    in_=gT0[:, fct, :],
                                    func=AF.Gelu_apprx_tanh,
                                    bias=bfc_sb[:, fct : fct + 1],
                                    scale=1.0,
                                )
                    emit_RS(3)
                    for i2 in range(2):
                        emit_E_tile(2, i2, se)
                    # proj(0); E(3) after the first proj tiles so its
                    # RS3-gated transposes don't head-of-line block the PE
                    for t4 in range(4):
                        for half in range(2):
                            emit_proj_tile(0, t4, half, sc, gT0, wp_get)
                        if t4 == 1:
                            emit_E_tile(3, 0, se)
                        if t4 == 2:
                            emit_E_tile(3, 1, se)
                    # fc(1) reusing gT0 + sf0 wf ring; then proj(1)
                    wf_tiles1 = {}
                    for fc4 in range(2):
                        wf_tiles1[fc4] = emit_fc_dma(fc4, sf0)
                    for fct in range(NFC):
                        nf = fct // 4 + 2
                        if fct % 4 == 0 and nf < NFC // 4:
                            wf_tiles1[nf] = emit_fc_dma(nf, sf0)
                        emit_fc_chain(1, fct, sf0, gT0, wf_tiles1)
                    for t4 in range(4):
                        for half in range(2):
                            emit_proj_tile(1, t4, half, sc, gT0, wp_get)

    _split_sync_waits(nc)
    return nc


def fc4_next(fct):
    return fct // 4 + 2


_NC_CACHE = None


def _get_nc():
    global _NC_CACHE
    if _NC_CACHE is None:
        _patch_tile_drain()
        _NC_CACHE = build_kernel()
    return _NC_CACHE


def make_in_maps(x, w_attn, w_o, ln1_g, ln1_b, ln2_g, ln2_b, w_fc, b_fc, w_proj, b_proj):
    bf = ml_dtypes.bfloat16
    e4 = ml_dtypes.float8_e4m3
    assert np.max(np.abs(np.asarray(ln1_b))) == 0.0, "ln1_b must be zero (not folded)"
    assert np.max(np.abs(np.asarray(ln2_b))) == 0.0, "ln2_b must be zero (not folded)"

    q_idx = np.arange(P)[None, :]
    k_idx = np.arange(P)[:, None]
    tri = (q_idx >= k_idx).astype(np.float32).astype(bf)
    sel = np.zeros((HL, HL * HD), np.float32)
    for hh in range(HL):
        sel[hh, hh * HD : (hh + 1) * HD] = 1.0
    sel = sel.astype(bf)
    sel2 = np.zeros((HD + 1, HL * HL), np.float32)
    for hh in range(HL):
        sel2[HD, hh * HL + hh] = 1.0
    sel2 = sel2.astype(bf)

    g1 = np.asarray(ln1_g, np.float32)[:, None]
    g2 = np.asarray(ln2_g, np.float32)[:, None]
    w_attn_f = np.asarray(w_attn, np.float32) * g1
    wfc_b = (np.asarray(w_fc, np.float32) * g2).astype(bf)
    wproj_b = np.asarray(w_proj, np.float32).astype(bf)

    wq = w_attn_f[:, :D]
    wk = w_attn_f[:, D : 2 * D]
    wv = w_attn_f[:, 2 * D :]

    in_maps = []
    for core in range(8):
        p, r = core // 2, core % 2
        hs = r * HL * HD
        wqk = np.concatenate([wq[:, hs : hs + 512], wk[:, hs : hs + 512]], axis=1)
        in_maps.append(
            {
                "x": np.ascontiguousarray(x[p]).astype(bf),
                "x_res": np.ascontiguousarray(
                    np.concatenate(
                        [
                            x[p, 512 * c + 256 * r : 512 * c + 256 * r + 256]
                            for c in range(4)
                        ],
                        axis=0,
                    ),
                    np.float32,
                ),
                "wqk": np.ascontiguousarray(wqk * S_W).astype(e4),
                "wv": np.ascontiguousarray(wv[:, hs : hs + 512] * S_W).astype(e4),
                "wo": np.ascontiguousarray(
                    np.asarray(w_o, np.float32)[hs : hs + 512, :] * S_WO
                ).astype(e4),
                "wfc": np.ascontiguousarray(wfc_b),
                "wproj": np.ascontiguousarray(wproj_b),
                "bfc": np.ascontiguousarray(b_fc, np.float32),
                "bproj": np.ascontiguousarray(b_proj, np.float32),
                "tri": tri,
                "sel": sel,
                "sel2": sel2,
            }
        )
    return in_maps


def kernel(**inputs):
    inputs = {k: np.asarray(v) for k, v in inputs.items()}
    nc = _get_nc()
    in_maps = make_in_maps(**inputs)
    res = run_bass_kernel_spmd(nc, in_maps, core_ids=list(range(8)))
    x = inputs["x"]
    B = x.shape[0]
    out = np.empty((B, T, D), np.float32)
    for core in range(8):
        p, r = core // 2, core % 2
        o = res.results[core]["out"]
        for c in range(4):
            out[p, 512 * c + 256 * r : 512 * c + 256 * r + 256] = o[
                c * 256 : (c + 1) * 256
            ]
    return out


if __name__ == "__main__":
    print("building...")
    nc = _get_nc()
    print("built:", sum(len(bb.instructions) for f in nc.m.functions for bb in f.blocks), "instructions")
